# revision 1
# baseline (speedup 1.0000x reference)
"""Trainium2 Bass kernel v4 for masked BasicBlock (conv3x3+BN+ReLU, gated, x2, residual).

Data-parallel over batch: 8 images -> 8 NeuronCores. Per core, NCHW [64,256,256]
in 8 row-strips of 32 output rows.

Core idea (v4): every conv tap (dy,dx) is ONE [128,512] matmul with a
block-diagonal lhsT — lower 64 partitions of the rhs hold rows for output
group A, upper 64 hold rows (shifted +2) for group B, and the 64x64 tap weight
matrix sits on both diagonal blocks. 9 taps + 1 gating-selector matmul per
4-row pair, for both convs. The selector (K=2, fp8) accumulates BIG*gmax into
conv1's PSUM (ReLU with bias-BIG*scale clamps inactive pixels to 0) and
broadcasts the gate for conv2's multiply.

conv1's ACT writes h straight into the H1 conv layout (two 64-partition
activations); two strip-level SBUF DMAs patch the cross-partition quarters.
Strips are software-pipelined; x in/out are bf16 on the wire.
"""
import sys

sys.path.insert(0, '/opt/trn_rl_repo')

import numpy as np
import ml_dtypes

BF16 = ml_dtypes.bfloat16
FP8 = ml_dtypes.float8_e4m3fn

B, C, H, W = 8, 64, 256, 256
WP = W + 2           # padded row width
R = 32               # output rows per strip
NS = H // R          # strips
NP1 = (R + 4) // 4   # conv1 pairs per strip (h rows r0-1 .. r0+34)
NP2 = R // 4         # conv2 pairs per strip
XR = R + 6           # x rows per strip: [r0-2, r0+36)
HR = R + 4           # h rows per strip: [r0-1, r0+35)
GPAD = 5             # gmax pad rows on top (1 mod 4 so pair groups are 4-row aligned)
GROWS = GPAD + H + 3 # 264 gmax padded rows
NSB = GROWS // 4     # 66 gmax super-blocks (4 rows = 2x512 blocks)
NSB2 = H // 4        # 64 gate super-blocks
BIG = 64.0

_CACHE = {}


def _build(iters=1):
    import concourse.bacc as bacc_mod
    import concourse.tile as tile
    import concourse.mybir as mybir

    dt = mybir.dt
    nc = bacc_mod.Bacc()

    x_d = nc.dram_tensor("x", [C, H, W], dt.bfloat16, kind="ExternalInput")
    gm2_d = nc.dram_tensor("gm2", [2, NSB * 512], dt.float8e4, kind="ExternalInput")
    gt2_d = nc.dram_tensor("gt2", [2, NSB2 * 512], dt.float8e4, kind="ExternalInput")
    wd1_d = nc.dram_tensor("wd1", [128, 9, 128], dt.bfloat16, kind="ExternalInput")
    wd2_d = nc.dram_tensor("wd2", [128, 9, 128], dt.bfloat16, kind="ExternalInput")
    sb1_d = nc.dram_tensor("sb1", [128, 2], dt.float32, kind="ExternalInput")
    sb2_d = nc.dram_tensor("sb2", [128, 2], dt.float32, kind="ExternalInput")
    selb_d = nc.dram_tensor("selb", [2, 128], dt.float8e4, kind="ExternalInput")
    selp_d = nc.dram_tensor("selp", [2, 128], dt.float8e4, kind="ExternalInput")
    o_d = nc.dram_tensor("o", [C, H, W], dt.bfloat16, kind="ExternalOutput")

    RELU = mybir.ActivationFunctionType.Relu
    IDENT = mybir.ActivationFunctionType.Identity

    with tile.TileContext(nc) as tc:
        with (
            tc.tile_pool(name="const", bufs=1) as cpool,
            tc.tile_pool(name="xs", bufs=4) as xpool,
            tc.tile_pool(name="hs", bufs=3) as hpool,
            tc.tile_pool(name="msk", bufs=3) as mpool,
            tc.tile_pool(name="ov", bufs=2) as ovpool,
            tc.tile_pool(name="work", bufs=2) as wpool,
            tc.tile_pool(name="ps1", bufs=2, space="PSUM") as ps1,
            tc.tile_pool(name="ps2", bufs=3, space="PSUM") as ps2,
            tc.tile_pool(name="pm", bufs=3, space="PSUM") as pmp,
        ):
            wd1 = cpool.tile([128, 9, 128], dt.bfloat16)
            wd2 = cpool.tile([128, 9, 128], dt.bfloat16)
            sb1 = cpool.tile([128, 2], dt.float32)
            sb2 = cpool.tile([128, 2], dt.float32)
            selb = cpool.tile([2, 128], dt.float8e4)
            selp = cpool.tile([2, 128], dt.float8e4)
            for t, d in ((wd1, wd1_d), (wd2, wd2_d), (sb1, sb1_d), (sb2, sb2_d),
                         (selb, selb_d), (selp, selp_d)):
                nc.sync.dma_start(t[:], d[:])
            warm = cpool.tile([2, 64], dt.bfloat16)
            nc.vector.memset(warm[:], 0)
            wps = ps2.tile([128, 512], dt.float32, tag="ps2")
            for i in range(80):
                off = (i % 8) * 64
                nc.tensor.matmul(wps[0:64, off:off + 64], warm[:, 0:64], warm[:, :],
                                 start=True, stop=True, tile_position=(0, 0), skip_group_check=True)

            def emit_load(s):
                r0 = s * R
                first = r0 - 2
                T1 = xpool.tile([128, XR, WP], dt.bfloat16, tag="T1")
                v0 = max(0, -first)
                v1 = min(XR, H - first)
                v1u = min(XR, H - first - 2)
                nc.vector.memset(T1[:, :, 0:1], 0)
                nc.vector.memset(T1[:, :, 257:258], 0)
                if v0 > 0:
                    nc.vector.memset(T1[0:64, 0:v0, :], 0)
                if v1 < XR:
                    nc.vector.memset(T1[0:64, v1:XR, :], 0)
                if v1u < XR:
                    nc.vector.memset(T1[64:128, v1u:XR, :], 0)
                if s == 0:
                    vh = 14
                    nc.gpsimd.dma_start(T1[0:64, v0:vh, 1:257], x_d[:, first + v0:first + vh, :])
                    nc.gpsimd.dma_start(T1[64:128, 0:vh, 1:257], x_d[:, first + 2:first + 2 + vh, :])
                    nc.gpsimd.dma_start(T1[0:64, vh:v1, 1:257], x_d[:, first + vh:first + v1, :])
                    nc.gpsimd.dma_start(T1[64:128, vh:v1u, 1:257], x_d[:, first + 2 + vh:first + 2 + v1u, :])
                else:
                    nc.gpsimd.dma_start(T1[0:64, v0:v1, 1:257], x_d[:, first + v0:first + v1, :])
                    nc.gpsimd.dma_start(T1[64:128, 0:v1u, 1:257], x_d[:, first + 2:first + 2 + v1u, :])
                gmw = mpool.tile([2, NP1 * 512], dt.float8e4, tag="gmw")
                gtw = mpool.tile([2, NP2 * 512], dt.float8e4, tag="gtw")
                S0 = r0 // 4 + 1
                nc.sync.dma_start(gmw[:], gm2_d[0:2, S0 * 512:(S0 + NP1) * 512])
                nc.sync.dma_start(gtw[:], gt2_d[0:2, (r0 // 4) * 512:(r0 // 4 + NP2) * 512])
                return T1, gmw, gtw

            def new_H1():
                H1 = hpool.tile([128, HR, WP], dt.bfloat16, tag="H1")
                nc.vector.memset(H1[:, :, 0:1], 0)
                nc.vector.memset(H1[:, :, 257:258], 0)
                return H1

            def emit_conv1(s, T1, gmw, H1s):
                # computes global pairs 8s+pq for pq in [0..8] (s==0) or [1..8];
                # the boundary pair (pq==8) also writes rows 0:4 of H1(s+1).
                H1 = H1s[s]
                for pq in (range(NP1) if s == 0 else range(1, NP1)):
                    acc = ps1.tile([128, 512], dt.float32, tag="ps1")
                    for k in range(9):
                        dy, dx = k // 3, k % 3
                        tt = 4 * pq + dy
                        nc.tensor.matmul(acc[:, :], wd1[:, k, :], T1[:, tt:tt + 2, dx:dx + 256],
                                         start=(k == 0), stop=False, tile_position=(0, 0),
                                         skip_group_check=True)
                    nc.tensor.matmul(acc[:, :], selb[:, :], gmw[0:2, pq * 512:pq * 512 + 512],
                                     start=False, stop=True, tile_position=(0, 0), skip_group_check=True)
                    accv = acc[:].rearrange("p (r w) -> p r w", r=2)
                    nc.scalar.activation(H1[0:64, 4 * pq:4 * pq + 2, 1:257], accv[0:64],
                                         RELU, bias=sb1[0:64, 1:2], scale=sb1[0:64, 0:1])
                    nc.scalar.activation(H1[64:128, 4 * pq:4 * pq + 2, 1:257], accv[64:128],
                                         RELU, bias=sb1[64:128, 1:2], scale=sb1[64:128, 0:1])
                    if pq == NP1 - 1 and s + 1 < NS:
                        H1n = new_H1()
                        H1s[s + 1] = H1n
                        nc.scalar.activation(H1n[0:64, 0:2, 1:257], accv[0:64],
                                             RELU, bias=sb1[0:64, 1:2], scale=sb1[0:64, 0:1])
                        nc.scalar.activation(H1n[64:128, 0:2, 1:257], accv[64:128],
                                             RELU, bias=sb1[64:128, 1:2], scale=sb1[64:128, 0:1])
                # patch the two cross-partition quarters:
                h1l4 = H1[0:64].rearrange("c (p a) w -> c p (a w)", a=4)
                h1u4 = H1[64:128].rearrange("c (p a) w -> c p (a w)", a=4)
                nc.sync.dma_start(h1l4[:, :, 2 * WP:4 * WP], h1u4[:, :, 0:2 * WP])
                nc.sync.dma_start(h1u4[:, 0:NP1 - 1, 2 * WP:4 * WP], h1l4[:, 1:NP1, 0:2 * WP])

            def emit_conv2(s, T1, H1, gtw):
                r0 = s * R
                OV = ovpool.tile([128, NP2, 512], dt.bfloat16, tag="OV")
                for q in range(NP2):
                    pm = pmp.tile([128, 512], dt.float32, tag="pm")
                    nc.tensor.matmul(pm[:, :], selp[:, :], gtw[0:2, q * 512:q * 512 + 512],
                                     start=True, stop=True, tile_position=(0, 0), skip_group_check=True)
                    acc2 = ps2.tile([128, 512], dt.float32, tag="ps2")
                    for k in range(9):
                        dy, dx = k // 3, k % 3
                        mm = 4 * q + dy
                        nc.tensor.matmul(acc2[:, :], wd2[:, k, :], H1[:, mm:mm + 2, dx:dx + 256],
                                         start=(k == 0), stop=(k == 8), tile_position=(0, 0),
                                         skip_group_check=True)
                    u2 = wpool.tile([128, 512], dt.bfloat16, tag="u2")
                    nc.vector.tensor_scalar(u2[:], acc2[:], sb2[:, 0:1], sb2[:, 1:2],
                                            mybir.AluOpType.mult, mybir.AluOpType.add)
                    t = wpool.tile([128, 512], dt.bfloat16, tag="t")
                    nc.vector.tensor_tensor(t[:], u2[:], pm[:], mybir.AluOpType.mult)
                    v = wpool.tile([128, 512], dt.bfloat16, tag="v")
                    lz = 4 * q + 2
                    nc.vector.tensor_tensor(v[:].rearrange("p (r w) -> p r w", r=2),
                                            t[:].rearrange("p (r w) -> p r w", r=2),
                                            T1[:, lz:lz + 2, 1:257], mybir.AluOpType.add)
                    nc.scalar.activation(OV[:, q, :], v[:], RELU)
                o4 = o_d[:, r0:r0 + R, :].rearrange("c (q a) w -> c q (a w)", a=4)
                if s == NS - 1:
                    for qa in range(0, NP2, 2):
                        nc.sync.dma_start(o4[:, qa:qa + 2, 0:512], OV[0:64, qa:qa + 2, :])
                        nc.sync.dma_start(o4[:, qa:qa + 2, 512:1024], OV[64:128, qa:qa + 2, :])
                else:
                    nc.sync.dma_start(o4[:, 0:4, 0:512], OV[0:64, 0:4, :])
                    nc.sync.dma_start(o4[:, 0:4, 512:1024], OV[64:128, 0:4, :])
                    nc.sync.dma_start(o4[:, 4:8, 0:512], OV[0:64, 4:8, :])
                    nc.sync.dma_start(o4[:, 4:8, 512:1024], OV[64:128, 4:8, :])

            for it in range(iters):
                T1s = {}
                H1s = {}
                T1s[0] = emit_load(0)
                T1s[1] = emit_load(1)
                H1s[0] = new_H1()
                emit_conv1(0, T1s[0][0], T1s[0][1], H1s)
                for s in range(NS):
                    if s + 2 < NS:
                        T1s[s + 2] = emit_load(s + 2)
                    if s + 1 < NS:
                        emit_conv1(s + 1, T1s[s + 1][0], T1s[s + 1][1], H1s)
                    emit_conv2(s, T1s[s][0], H1s[s], T1s[s][2])
                    T1s.pop(s)
                    H1s.pop(s)
    nc.finalize()
    return nc


def _host_prep(x, gate, w1, scale1, bias1, w2, scale2, bias2):
    # wd[k]: block-diagonal [128,128], diag blocks = wt[:, :, dy, dx] (tap k = 3*dy+dx)
    def pack(w):
        wt = np.transpose(w, (1, 0, 2, 3))  # [ci, co, dy, dx]
        wd = np.zeros((128, 9, 128), np.float32)
        for k in range(9):
            dy, dx = k // 3, k % 3
            wd[0:64, k, 0:64] = wt[:, :, dy, dx]
            wd[64:128, k, 64:128] = wt[:, :, dy, dx]
        return wd.astype(BF16)

    wd1 = pack(w1)
    wd2 = pack(w2)
    # conv1 bias folded with the -BIG gate clamp: relu(s*(acc + BIG*g) + b - s*BIG)
    sb1 = np.stack([np.tile(scale1, 2), np.tile(bias1 - scale1 * BIG, 2)], axis=1).astype(np.float32)
    sb2 = np.stack([np.tile(scale2, 2), np.tile(bias2, 2)], axis=1).astype(np.float32)

    selb = np.zeros((2, 128), np.float32)
    selb[0, 0:64] = BIG
    selb[1, 64:128] = BIG
    selp = np.zeros((2, 128), np.float32)
    selp[0, 0:64] = 1.0
    selp[1, 64:128] = 1.0
    selb = selb.astype(FP8)
    selp = selp.astype(FP8)

    g = gate[:, 0]                                   # [B, H, W]
    gp = np.pad(g, ((0, 0), (1, 1), (1, 1)))
    gm = np.zeros_like(g)
    for dy in range(3):
        for dx in range(3):
            np.maximum(gm, gp[:, dy:dy + H, dx:dx + W], out=gm)

    def blocks2(padded):                             # [rows(4k), W] -> [2, k*512]
        nsb = padded.shape[0] // 4
        arr = padded.reshape(nsb, 2, 512)
        return np.ascontiguousarray(arr.transpose(1, 0, 2)).reshape(2, -1).astype(FP8)

    gm2_l, gt2_l, xbf = [], [], []
    for bi in range(B):
        gmp = np.zeros((GROWS, W), np.float32)
        gmp[GPAD:GPAD + H] = gm[bi]
        gm2_l.append(blocks2(gmp))
        gt2_l.append(blocks2(g[bi]))
        xbf.append(np.ascontiguousarray(x[bi]).astype(BF16))
    return dict(wd1=wd1, wd2=wd2, sb1=sb1, sb2=sb2,
                selb=selb, selp=selp, gm2=gm2_l, gt2=gt2_l, x=xbf)


def _in_map(prep, bi):
    return {
        "x": prep["x"][bi],
        "gm2": prep["gm2"][bi], "gt2": prep["gt2"][bi],
        "wd1": prep["wd1"], "wd2": prep["wd2"],
        "sb1": prep["sb1"], "sb2": prep["sb2"], "selb": prep["selb"], "selp": prep["selp"],
    }


def kernel(x, gate, w1, scale1, bias1, w2, scale2, bias2):
    from concourse.bass_utils import run_bass_kernel_spmd

    x = np.asarray(x, np.float32)
    gate = np.asarray(gate, np.float32)
    prep = _host_prep(x, gate, np.asarray(w1, np.float32), np.asarray(scale1, np.float32),
                      np.asarray(bias1, np.float32), np.asarray(w2, np.float32),
                      np.asarray(scale2, np.float32), np.asarray(bias2, np.float32))

    if 'nc' not in _CACHE:
        _CACHE['nc'] = _build()
    nc = _CACHE['nc']

    in_maps = [_in_map(prep, bi) for bi in range(B)]
    res = run_bass_kernel_spmd(nc, in_maps, core_ids=list(range(B)))
    _CACHE['last_results'] = res
    out = np.stack([res.results[bi]["o"].astype(np.float32) for bi in range(B)], axis=0)
    return out



# revision 8
# speedup vs baseline: 1.3518x; 1.3518x over previous
"""Trainium2 Bass kernel v8 for masked BasicBlock (conv3x3+BN+ReLU, gated, x2, residual).

Data-parallel over batch: 8 images -> 8 NeuronCores. Per core, NCHW [64,256,256]
in 8 row-strips of 32 output rows.

v8: fp8e4 DoubleRow matmuls with full hi/lo error compensation.
Every conv tap is expressed as 3 plane-products (Wh*xh, Wh*xl, Wl*xh) where
xh=fp8(x), xl=fp8(x-xh), Wh=fp8(16w), Wl=fp8(256(w-Wh/16))/16. 27 products +
1 gate/bias plane pack into 14 DoubleRow matmuls (2 planes each) per 256-col
half, out [128,256] = (row t | groups A,B) with the block-diagonal lhsT trick
(partition halves hold x and x-shifted-by-2-rows). PSUM accumulates 16*conv;
the conv1 gate rides a selector plane (448*gmax, relu-clamp via folded bias),
conv2's bias rides a ones-row plane.

h is split on-chip: ACT writes Y (bf16) + Hhi (fp8), DVE writes Hlo = Y - Hhi.
conv2 post-chain on DVE: t=(acc*s2/16)*gate_bcast (fused), v=t+x_res, relu.
Residual x / gate / output use 128-partition packed [*,q,512] DRAM layouts.
"""
import sys

sys.path.insert(0, '/opt/trn_rl_repo')

import numpy as np
import ml_dtypes

BF16 = ml_dtypes.bfloat16
FP8 = ml_dtypes.float8_e4m3      # concourse float8e4 (IEEE e4m3, max finite 240)

B, C, H, W = 8, 64, 256, 256
WP = 258             # padded row width
R = 32               # output rows per strip
NS = H // R          # strips
XRH = 38             # x rows per strip section [r0-2, r0+36)
XLO = 38             # xl section row offset inside strip region
GOFF = 76            # gate section row offset
SRO = 96             # strip region rows in xpack (38+38+18+2 pad)
HR = 36              # H1 rows per section [r0-1, r0+35)
HLOFF = 36           # H1 lo section offset
HONES = 72           # H1 ones row
H1R = 73             # H1 total rows
NQ = 64              # 4-row q-blocks over H
BIGW = 240.0         # gate selector weight (clamp = 15*s1, max finite e4m3)
KW = 16.0            # weight scale

_CACHE = {}


def _build():
    import concourse.bacc as bacc_mod
    import concourse.tile as tile
    import concourse.mybir as mybir
    import concourse.bass as bass

    dt = mybir.dt
    DR = mybir.MatmulPerfMode.DoubleRow
    ALU = mybir.AluOpType
    RELU = mybir.ActivationFunctionType.Relu
    nc = bacc_mod.Bacc()

    xp_d = nc.dram_tensor("xp", [128, NS * SRO, WP], dt.float8e4, kind="ExternalInput")
    wd1_d = nc.dram_tensor("wd1", [128, 14, 2, 128], dt.float8e4, kind="ExternalInput")
    wd2_d = nc.dram_tensor("wd2", [128, 14, 2, 128], dt.float8e4, kind="ExternalInput")
    sb1_d = nc.dram_tensor("sb1", [128, 2], dt.float32, kind="ExternalInput")
    sb2_d = nc.dram_tensor("sb2", [128, 1], dt.float32, kind="ExternalInput")
    xr_d = nc.dram_tensor("xr", [128, NQ, 512], dt.bfloat16, kind="ExternalInput")
    gt_d = nc.dram_tensor("gt", [128, NQ, 512], dt.float8e4, kind="ExternalInput")
    o_d = nc.dram_tensor("o", [128, NQ, 512], dt.bfloat16, kind="ExternalOutput")

    # conv tap plane-pair schedule: (flat-offset lambda, plane step) per DR matmul.
    # Offsets are relative to the (unit,half) base row bt in the hi section;
    # taps k=0..8 (dy=k//3, dx=k%3).  A/B = Wh x (hi/lo), C = Wl x hi.
    def tap_off(bt, dy, dx, sec):
        return (sec + bt + dy) * WP + dx

    with tile.TileContext(nc) as tc:
        with (
            tc.tile_pool(name="const", bufs=1) as cpool,
            tc.tile_pool(name="xs", bufs=3) as xpool,
            tc.tile_pool(name="hs", bufs=3) as hpool,
            tc.tile_pool(name="ys", bufs=3) as ypool,
            tc.tile_pool(name="rg", bufs=2) as rgpool,
            tc.tile_pool(name="ov", bufs=2) as ovpool,
            tc.tile_pool(name="wk", bufs=4) as wpool,
            tc.tile_pool(name="ps1", bufs=2, space="PSUM") as ps1,
            tc.tile_pool(name="ps2", bufs=2, space="PSUM") as ps2,
            tc.tile_pool(name="psw", bufs=1, space="PSUM") as psw,
        ):
            wd1 = cpool.tile([128, 14, 2, 128], dt.float8e4)
            wd2 = cpool.tile([128, 14, 2, 128], dt.float8e4)
            sb1 = cpool.tile([128, 2], dt.float32)
            sb2 = cpool.tile([128, 1], dt.float32)
            for t, d in ((wd1, wd1_d), (wd2, wd2_d), (sb1, sb1_d), (sb2, sb2_d)):
                nc.sync.dma_start(t[:], d[:])

            # PE p-state warmup: cheap DR matmuls on a zeroed tile while DMAs run.
            warm = cpool.tile([128, 2, 128], dt.float8e4)
            nc.vector.memset(warm[:], 0)
            wps = psw.tile([128, 128], dt.float32)
            for _ in range(80):
                nc.tensor.matmul(wps[:, :], warm[:, :, :], warm[:, :, :],
                                 start=True, stop=True, perf_mode=DR,
                                 tile_position=(0, 0), skip_group_check=True)

            def pap(base, off, step):
                # [128, 2, 256] plane-pair AP at flat free offset `off`
                ps = base.ap[0][0]
                return bass.AP(base.tensor, base.offset + off,
                               [[ps, 128], [step, 2], [1, 256]])

            def conv_paps(base, bt, sec_lo, sel_off):
                # 14 (rhs AP) entries for one 256-col half with base row bt
                out = []
                for k in range(9):
                    o0 = tap_off(bt, k // 3, k % 3, 0)
                    out.append((o0, sec_lo * WP))
                cpairs = [((0, 0), (1, 0)), ((0, 1), (2, 0)), ((1, 1), (2, 1)),
                          ((0, 2), (1, 2))]
                for (dy0, dx0), (dy1, dx1) in cpairs:
                    o0 = tap_off(bt, dy0, dx0, 0)
                    o1 = tap_off(bt, dy1, dx1, 0)
                    out.append((o0, o1 - o0))
                o0 = tap_off(bt, 2, 2, 0)
                out.append((o0, sel_off - o0))
                return [pap(base, o0, st) for o0, st in out]

            def emit_load(s):
                T1 = xpool.tile([128, SRO, WP], dt.float8e4, tag="T1")
                nc.sync.dma_start(T1[:], xp_d[:, s * SRO:(s + 1) * SRO, :])
                return T1

            def new_H1():
                H1 = hpool.tile([128, H1R, WP], dt.float8e4, tag="H1")
                nc.vector.memset(H1[:, 0:H1R, 0:1], 0)
                nc.vector.memset(H1[:, 0:H1R, 257:258], 0)
                nc.vector.memset(H1[:, HONES, :], 1.0)
                return H1

            def emit_conv1(s, T1, H1, H1n):
                # units u cover h rows (A: r0-1+4u, r0+4u; B: +2).
                # strip 0 runs u=0..8; steady strips u=1..8 (rows 0:4 of H1 were
                # written by the previous strip's boundary unit).
                T1b = T1[:]
                u0 = 0 if s == 0 else 1
                for u in range(u0, 9):
                    acc = ps1.tile([128, 2, 256], dt.float32, tag="ps1")
                    for h in range(2):
                        bt = 4 * u + h
                        paps = conv_paps(T1b, bt, XLO, (GOFF + 2 * u + h) * WP)
                        for j in range(14):
                            nc.tensor.matmul(acc[:, h, :], wd1[:, j, :, :], paps[j],
                                             start=(j == 0), stop=(j == 13),
                                             perf_mode=DR, tile_position=(0, 0),
                                             skip_group_check=True)
                    Y = ypool.tile([128, 2, 256], dt.bfloat16, tag="Y")
                    nc.scalar.activation(Y[:], acc[:], RELU,
                                         bias=sb1[:, 1:2], scale=sb1[:, 0:1])
                    sl = 4 * u
                    nc.scalar.copy(H1[:, sl:sl + 2, 1:257], Y[:])
                    nc.vector.scalar_tensor_tensor(
                        H1[:, HLOFF + sl:HLOFF + sl + 2, 1:257], Y[:], 1.0,
                        H1[:, sl:sl + 2, 1:257], ALU.mult, ALU.subtract)
                    if u == 8 and H1n is not None:
                        nc.scalar.copy(H1n[:, 0:2, 1:257], Y[:])
                        nc.vector.scalar_tensor_tensor(
                            H1n[:, HLOFF:HLOFF + 2, 1:257], Y[:], 1.0,
                            H1n[:, 0:2, 1:257], ALU.mult, ALU.subtract)
                # cross-partition quarter patches per section:
                # lower rows {4b+2,4b+3} <- upper rows {4b,4b+1} (b=0..8) and
                # upper rows {4b+2,4b+3} <- lower rows {4(b+1),4(b+1)+1} (b=0..7)
                for sec in (0, HLOFF):
                    lo = H1[0:64, sec:sec + HR, :].rearrange("c (b a) w -> c b (a w)", a=4)
                    up = H1[64:128, sec:sec + HR, :].rearrange("c (b a) w -> c b (a w)", a=4)
                    nc.gpsimd.dma_start(lo[:, 0:9, 2 * WP:4 * WP], up[:, 0:9, 0:2 * WP])
                    nc.gpsimd.dma_start(up[:, 0:8, 2 * WP:4 * WP], lo[:, 1:9, 0:2 * WP])

            def emit_conv2(s, H1, xr, gt):
                H1b = H1[:]
                OV = ovpool.tile([128, 8, 512], dt.bfloat16, tag="OV")
                for q in range(8):
                    acc2 = ps2.tile([128, 2, 256], dt.float32, tag="ps2")
                    for h in range(2):
                        bt = 4 * q + h
                        paps = conv_paps(H1b, bt, HLOFF, HONES * WP)
                        for j in range(14):
                            nc.tensor.matmul(acc2[:, h, :], wd2[:, j, :, :], paps[j],
                                             start=(j == 0), stop=(j == 13),
                                             perf_mode=DR, tile_position=(0, 0),
                                             skip_group_check=True)
                    t = wpool.tile([128, 512], dt.bfloat16, tag="t")
                    nc.vector.scalar_tensor_tensor(
                        t[:], acc2[:].rearrange("p a b -> p (a b)"), sb2[:, 0:1],
                        gt[:, q, :], ALU.mult, ALU.mult)
                    v = wpool.tile([128, 512], dt.bfloat16, tag="v")
                    nc.vector.tensor_tensor(v[:], t[:], xr[:, q, :], ALU.add)
                    nc.vector.tensor_scalar(OV[:, q, :], v[:], 0.0, None, ALU.max)
                nc.gpsimd.dma_start(o_d[:, 8 * s:8 * s + 8, :], OV[:])

            def load_rg(s):
                xr = rgpool.tile([128, 8, 512], dt.bfloat16, tag="xr")
                gt = rgpool.tile([128, 8, 512], dt.float8e4, tag="gt")
                nc.sync.dma_start(xr[:], xr_d[:, 8 * s:8 * s + 8, :])
                nc.sync.dma_start(gt[:], gt_d[:, 8 * s:8 * s + 8, :])
                return xr, gt

            T1s = {0: emit_load(0), 1: emit_load(1)}
            H1s = {0: new_H1(), 1: new_H1()}
            rgs = {0: load_rg(0)}
            emit_conv1(0, T1s[0], H1s[0], H1s[1])
            for s in range(NS):
                if s + 2 < NS:
                    T1s[s + 2] = emit_load(s + 2)
                    H1s[s + 2] = new_H1()
                if s + 1 < NS:
                    rgs[s + 1] = load_rg(s + 1)
                    emit_conv1(s + 1, T1s[s + 1], H1s[s + 1], H1s.get(s + 2))
                emit_conv2(s, H1s[s], *rgs[s])
                T1s.pop(s, None)
                H1s.pop(s, None)
                rgs.pop(s, None)
    nc.finalize()
    return nc


def _host_prep(x, gate, w1, scale1, bias1, w2, scale2, bias2):
    # ---- weights: Wh = fp8(16w), Wl16 = fp8(256(w - Wh/16))/16, block-diag ----
    def pack_w(w, extra):
        wt = np.transpose(w, (1, 0, 2, 3)).astype(np.float32)  # [ci, co, dy, dx]
        wh8 = (wt * KW).astype(FP8)
        wh = wh8.astype(np.float32)
        wl8 = ((wt * KW - wh) * KW).astype(FP8)
        wl16 = (wl8.astype(np.float32) / KW).astype(FP8).astype(np.float32)
        wh, wl16 = wh8.astype(np.float32), wl16

        def bd(m):  # [64,64] -> block-diag [128,128]
            out = np.zeros((128, 128), np.float32)
            out[0:64, 0:64] = m
            out[64:128, 64:128] = m
            return out

        wd = np.zeros((128, 14, 2, 128), np.float32)
        for k in range(9):
            dy, dx = k // 3, k % 3
            m = bd(wh[:, :, dy, dx])
            wd[:, k, 0, :] = m
            wd[:, k, 1, :] = m
        cpairs = [((0, 0), (1, 0)), ((0, 1), (2, 0)), ((1, 1), (2, 1)),
                  ((0, 2), (1, 2))]
        for j, ((dy0, dx0), (dy1, dx1)) in enumerate(cpairs):
            wd[:, 9 + j, 0, :] = bd(wl16[:, :, dy0, dx0])
            wd[:, 9 + j, 1, :] = bd(wl16[:, :, dy1, dx1])
        wd[:, 13, 0, :] = bd(wl16[:, :, 2, 2])
        wd[:, 13, 1, :] = extra
        return wd.astype(FP8)

    wsel = np.zeros((128, 128), np.float32)
    wsel[0, 0:64] = BIGW
    wsel[1, 64:128] = BIGW
    s2 = scale2.astype(np.float32)
    wbias = np.zeros((128, 128), np.float32)
    wbias[0, 0:64] = KW * bias2 / s2
    wbias[1, 64:128] = KW * bias2 / s2
    wd1 = pack_w(w1, wsel)
    wd2 = pack_w(w2, wbias)

    sb1 = np.stack([np.tile(scale1 / KW, 2),
                    np.tile(bias1 - (BIGW / KW) * scale1, 2)], axis=1).astype(np.float32)
    sb2v = np.tile(s2 / KW, 2)[:, None].astype(np.float32)

    # ---- gate / gmax ----
    g = gate[:, 0]                                   # [B, H, W]
    gp = np.pad(g, ((0, 0), (1, 1), (1, 1)))
    gm = np.zeros_like(g)
    for dy in range(3):
        for dx in range(3):
            np.maximum(gm, gp[:, dy:dy + H, dx:dx + W], out=gm)

    xh8 = x.astype(FP8)                              # [B, C, H, W]
    xl8 = (x - xh8.astype(np.float32)).astype(FP8)

    xpack_l, xr_l, gt_l, = [], [], []
    # padded row lookup: prow = row + 2, rows [-2, 262)
    for bi in range(B):
        xhp = np.zeros((64, 264, WP), FP8)
        xlp = np.zeros((64, 264, WP), FP8)
        xhp[:, 2:2 + H, 1:257] = xh8[bi]
        xlp[:, 2:2 + H, 1:257] = xl8[bi]
        gmp = np.zeros((264, W), np.float32)
        gmp[2:2 + H] = gm[bi]
        xpack = np.zeros((128, NS * SRO, WP), FP8)
        for s in range(NS):
            r0 = R * s
            b0 = s * SRO
            xpack[0:64, b0:b0 + XRH] = xhp[:, r0:r0 + XRH]
            xpack[64:128, b0:b0 + XRH] = xhp[:, r0 + 2:r0 + 2 + XRH]
            xpack[0:64, b0 + XLO:b0 + XLO + XRH] = xlp[:, r0:r0 + XRH]
            xpack[64:128, b0 + XLO:b0 + XLO + XRH] = xlp[:, r0 + 2:r0 + 2 + XRH]
            for u in range(9):
                for hh in range(2):
                    t = r0 - 1 + 4 * u + hh
                    row = b0 + GOFF + 2 * u + hh
                    xpack[0, row, 0:256] = gmp[t + 2].astype(FP8)
                    xpack[1, row, 0:256] = gmp[t + 4].astype(FP8)
        xpack_l.append(xpack)

        xv = x[bi].reshape(64, NQ, 4, 256)
        xr = np.empty((128, NQ, 512), BF16)
        xr[0:64] = xv[:, :, 0:2].reshape(64, NQ, 512)
        xr[64:128] = xv[:, :, 2:4].reshape(64, NQ, 512)
        xr_l.append(xr)

        gv = g[bi].reshape(NQ, 4, 256)
        gt = np.empty((128, NQ, 512), FP8)
        gt[0:64] = np.broadcast_to(gv[:, 0:2].reshape(1, NQ, 512), (64, NQ, 512))
        gt[64:128] = np.broadcast_to(gv[:, 2:4].reshape(1, NQ, 512), (64, NQ, 512))
        gt_l.append(gt)

    return dict(wd1=wd1, wd2=wd2, sb1=sb1, sb2=sb2v,
                xp=xpack_l, xr=xr_l, gt=gt_l)


def _in_map(prep, bi):
    return {
        "xp": prep["xp"][bi], "xr": prep["xr"][bi], "gt": prep["gt"][bi],
        "wd1": prep["wd1"], "wd2": prep["wd2"],
        "sb1": prep["sb1"], "sb2": prep["sb2"],
    }


def kernel(x, gate, w1, scale1, bias1, w2, scale2, bias2):
    from concourse.bass_utils import run_bass_kernel_spmd

    x = np.asarray(x, np.float32)
    gate = np.asarray(gate, np.float32)
    prep = _host_prep(x, gate, np.asarray(w1, np.float32), np.asarray(scale1, np.float32),
                      np.asarray(bias1, np.float32), np.asarray(w2, np.float32),
                      np.asarray(scale2, np.float32), np.asarray(bias2, np.float32))

    if 'nc' not in _CACHE:
        _CACHE['nc'] = _build()
    nc = _CACHE['nc']

    in_maps = [_in_map(prep, bi) for bi in range(B)]
    res = run_bass_kernel_spmd(nc, in_maps, core_ids=list(range(B)))
    _CACHE['last_results'] = res
    out = np.empty((B, C, H, W), np.float32)
    for bi in range(B):
        ov = res.results[bi]["o"].astype(np.float32)    # [128, NQ, 512]
        ov4 = ov.reshape(128, NQ, 2, 256)
        out[bi, :, 0::4, :] = ov4[0:64, :, 0, :].transpose(0, 1, 2)
        out[bi, :, 1::4, :] = ov4[0:64, :, 1, :]
        out[bi, :, 2::4, :] = ov4[64:128, :, 0, :]
        out[bi, :, 3::4, :] = ov4[64:128, :, 1, :]
    return out


# revision 30
# speedup vs baseline: 1.6221x; 1.2000x over previous
"""Trainium2 Bass kernel v8 for masked BasicBlock (conv3x3+BN+ReLU, gated, x2, residual).

Data-parallel over batch: 8 images -> 8 NeuronCores. Per core, NCHW [64,256,256]
in 8 row-strips of 32 output rows.

v8: fp8e4 DoubleRow matmuls with full hi/lo error compensation.
Every conv tap is expressed as 3 plane-products (Wh*xh, Wh*xl, Wl*xh) where
xh=fp8(x), xl=fp8(x-xh), Wh=fp8(16w), Wl=fp8(256(w-Wh/16))/16. 27 products +
1 gate/bias plane pack into 14 DoubleRow matmuls (2 planes each) per 256-col
half, out [128,256] = (row t | groups A,B) with the block-diagonal lhsT trick
(partition halves hold x and x-shifted-by-2-rows). PSUM accumulates 16*conv;
the conv1 gate rides a selector plane (448*gmax, relu-clamp via folded bias),
conv2's bias rides a ones-row plane.

h is split on-chip: ACT writes Y (bf16) + Hhi (fp8), DVE writes Hlo = Y - Hhi.
conv2 post-chain on DVE: t=(acc*s2/16)*gate_bcast (fused), v=t+x_res, relu.
Residual x / gate / output use 128-partition packed [*,q,512] DRAM layouts.
"""
import sys

sys.path.insert(0, '/opt/trn_rl_repo')

import numpy as np
import ml_dtypes

BF16 = ml_dtypes.bfloat16
FP8 = ml_dtypes.float8_e4m3      # concourse float8e4 (IEEE e4m3, max finite 240)

B, C, H, W = 8, 64, 256, 256
WP = 258             # padded row width
R = 32               # output rows per strip
NS = H // R          # strips
XRH = 38             # x rows per strip section [r0-2, r0+36)
XLO = 38             # xl section row offset inside strip region
GOFF = 76            # gate section row offset
SRO = 96             # strip region rows in xpack (38+38+18+2 pad)
HR = 36              # H1 rows per section [r0-1, r0+35)
HLOFF = 36           # H1 lo section offset
HONES = 72           # H1 ones row
H1R = 73             # H1 total rows
NQ = 64              # 4-row q-blocks over H
BIGW = 240.0         # gate selector weight (clamp = 15*s1, max finite e4m3)
KW = 16.0            # weight scale

_CACHE = {}


def _build():
    import concourse.bacc as bacc_mod
    import concourse.tile as tile
    import concourse.mybir as mybir
    import concourse.bass as bass

    dt = mybir.dt
    DR = mybir.MatmulPerfMode.DoubleRow
    ALU = mybir.AluOpType
    RELU = mybir.ActivationFunctionType.Relu
    nc = bacc_mod.Bacc()

    xp_d = nc.dram_tensor("xp", [128, NS * SRO, WP], dt.float8e4, kind="ExternalInput")
    wd1_d = nc.dram_tensor("wd1", [128, 14, 2, 128], dt.float8e4, kind="ExternalInput")
    wd2_d = nc.dram_tensor("wd2", [128, 11, 2, 128], dt.float8e4, kind="ExternalInput")
    sb1_d = nc.dram_tensor("sb1", [128, 2], dt.float32, kind="ExternalInput")
    sb2_d = nc.dram_tensor("sb2", [128, 1], dt.float32, kind="ExternalInput")
    xr_d = nc.dram_tensor("xr", [128, NQ, 512], dt.bfloat16, kind="ExternalInput")
    gt_d = nc.dram_tensor("gt", [128, NQ, 512], dt.float8e4, kind="ExternalInput")
    o_d = nc.dram_tensor("o", [128, NQ, 512], dt.bfloat16, kind="ExternalOutput")

    # conv tap plane-pair schedule: (flat-offset lambda, plane step) per DR matmul.
    # Offsets are relative to the (unit,half) base row bt in the hi section;
    # taps k=0..8 (dy=k//3, dx=k%3).  A/B = Wh x (hi/lo), C = Wl x hi.
    def tap_off(bt, dy, dx, sec):
        return (sec + bt + dy) * WP + dx

    with tile.TileContext(nc) as tc:
        with (
            tc.tile_pool(name="const", bufs=1) as cpool,
            tc.tile_pool(name="xs", bufs=3) as xpool,
            tc.tile_pool(name="hs", bufs=3) as hpool,
            tc.tile_pool(name="ys", bufs=3) as ypool,
            tc.tile_pool(name="rg", bufs=2) as rgpool,
            tc.tile_pool(name="ov", bufs=2) as ovpool,
            tc.tile_pool(name="wk", bufs=4) as wpool,
            tc.tile_pool(name="ps1", bufs=2, space="PSUM") as ps1,
            tc.tile_pool(name="ps2", bufs=2, space="PSUM") as ps2,
            tc.tile_pool(name="psw", bufs=1, space="PSUM") as psw,
        ):
            wd1 = cpool.tile([128, 14, 2, 128], dt.float8e4)
            wd2 = cpool.tile([128, 11, 2, 128], dt.float8e4)
            sb1 = cpool.tile([128, 2], dt.float32)
            sb2 = cpool.tile([128, 1], dt.float32)
            nc.scalar.dma_start(wd1[:], wd1_d[:])

            # PE p-state warmup: cheap DR matmuls on a zeroed tile while DMAs run.
            warm = cpool.tile([128, 2, 128], dt.float8e4)
            nc.vector.memset(warm[:], 0)
            wps = psw.tile([128, 128], dt.float32)
            for _ in range(110):
                nc.tensor.matmul(wps[0:64, 0:64], warm[:, :, 0:64], warm[:, :, 0:64],
                                 start=True, stop=True, perf_mode=DR,
                                 tile_position=(0, 0), skip_group_check=True)

            def pap(base, off, step):
                # [128, 2, 256] plane-pair AP at flat free offset `off`
                ps = base.ap[0][0]
                return bass.AP(base.tensor, base.offset + off,
                               [[ps, 128], [step, 2], [1, 256]])

            def conv_paps(base, bt, sec_lo, sel_off, cpairs):
                # rhs AP entries for one 256-col half with base row bt:
                # 9 (Wh x hi, Wh x lo) pairs, then C-pairs, then (C22, sel/bias)
                out = []
                for k in range(9):
                    o0 = tap_off(bt, k // 3, k % 3, 0)
                    out.append((o0, sec_lo * WP))
                for (dy0, dx0), (dy1, dx1) in cpairs:
                    o0 = tap_off(bt, dy0, dx0, 0)
                    o1 = tap_off(bt, dy1, dx1, 0)
                    out.append((o0, o1 - o0))
                o0 = tap_off(bt, 2, 2, 0)
                out.append((o0, sel_off - o0))
                return [pap(base, o0, st) for o0, st in out]

            CP1 = [((0, 0), (1, 0)), ((0, 1), (2, 0)), ((1, 1), (2, 1)),
                   ((0, 2), (1, 2))]          # conv1: full w-lo coverage, 14 DR
            CP2 = [((0, 2), (1, 2))]          # conv2: partial w-lo, 11 DR

            def emit_load(s, split=False):
                T1 = xpool.tile([128, SRO, WP], dt.float8e4, tag="T1")
                b0 = s * SRO
                if split:
                    # first strip: need-ordered chunks across queues so the rows
                    # unit 0 needs (xh 0:6, xl 38:44, gates 76:78) land first
                    order = [(nc.sync, 0, 8), (nc.gpsimd, 38, 46), (nc.scalar, 76, 84),
                             (nc.sync, 8, 24), (nc.gpsimd, 46, 64), (nc.scalar, 84, 96),
                             (nc.sync, 24, 38), (nc.gpsimd, 64, 76)]
                    for eng, r0, r1 in order:
                        eng.dma_start(T1[:, r0:r1, :], xp_d[:, b0 + r0:b0 + r1, :])
                else:
                    nc.sync.dma_start(T1[:], xp_d[:, b0:b0 + SRO, :])
                return T1

            def new_H1():
                H1 = hpool.tile([128, H1R, WP], dt.float8e4, tag="H1")
                nc.vector.memset(H1[:, 0:H1R, 0:1], 0)
                nc.vector.memset(H1[:, 0:H1R, 257:258], 0)
                nc.vector.memset(H1[:, HONES, :], 1.0)
                return H1

            def emit_conv1(s, T1, H1, H1n):
                # units u cover h rows (A: r0-1+4u, r0+4u; B: +2).
                # strip 0 runs u=0..8; steady strips u=1..8 (rows 0:4 of H1 were
                # written by the previous strip's boundary unit).
                T1b = T1[:]
                u0 = 0 if s == 0 else 1
                for u in range(u0, 9):
                    acc = ps1.tile([128, 2, 256], dt.float32, tag="ps1")
                    for h in range(2):
                        bt = 4 * u + h
                        paps = conv_paps(T1b, bt, XLO, (GOFF + 2 * u + h) * WP, CP1)
                        for j in range(14):
                            nc.tensor.matmul(acc[:, h, :], wd1[:, j, :, :], paps[j],
                                             start=(j == 0), stop=(j == 13),
                                             perf_mode=DR, tile_position=(0, 0),
                                             skip_group_check=True)
                    Y = ypool.tile([128, 2, 256], dt.bfloat16, tag="Y")
                    nc.scalar.activation(Y[:], acc[:], RELU,
                                         bias=sb1[:, 1:2], scale=sb1[:, 0:1])
                    sl = 4 * u
                    nc.scalar.copy(H1[:, sl:sl + 2, 1:257], Y[:])
                    nc.vector.scalar_tensor_tensor(
                        H1[:, HLOFF + sl:HLOFF + sl + 2, 1:257], Y[:], 1.0,
                        H1[:, sl:sl + 2, 1:257], ALU.mult, ALU.subtract)
                    if u == 8 and H1n is not None:
                        nc.scalar.copy(H1n[:, 0:2, 1:257], Y[:])
                        nc.vector.scalar_tensor_tensor(
                            H1n[:, HLOFF:HLOFF + 2, 1:257], Y[:], 1.0,
                            H1n[:, 0:2, 1:257], ALU.mult, ALU.subtract)
                # cross-partition quarter patches per section:
                # lower rows {4b+2,4b+3} <- upper rows {4b,4b+1} (b=0..8) and
                # upper rows {4b+2,4b+3} <- lower rows {4(b+1),4(b+1)+1} (b=0..7)
                for sec in (0, HLOFF):
                    lo = H1[0:64, sec:sec + HR, :].rearrange("c (b a) w -> c b (a w)", a=4)
                    up = H1[64:128, sec:sec + HR, :].rearrange("c (b a) w -> c b (a w)", a=4)
                    nc.gpsimd.dma_start(lo[:, 0:9, 2 * WP:4 * WP], up[:, 0:9, 0:2 * WP])
                    nc.gpsimd.dma_start(up[:, 0:8, 2 * WP:4 * WP], lo[:, 1:9, 0:2 * WP])

            def emit_conv2(s, H1, xr, gt):
                H1b = H1[:]
                OV = ovpool.tile([128, 8, 512], dt.bfloat16, tag="OV")
                for q in range(8):
                    acc2 = ps2.tile([128, 2, 256], dt.float32, tag="ps2")
                    for h in range(2):
                        bt = 4 * q + h
                        paps = conv_paps(H1b, bt, HLOFF, HONES * WP, CP2)
                        for j in range(11):
                            nc.tensor.matmul(acc2[:, h, :], wd2[:, j, :, :], paps[j],
                                             start=(j == 0), stop=(j == 10),
                                             perf_mode=DR, tile_position=(0, 0),
                                             skip_group_check=True)
                    t = wpool.tile([128, 512], dt.bfloat16, tag="t")
                    nc.vector.scalar_tensor_tensor(
                        t[:], acc2[:].rearrange("p a b -> p (a b)"), sb2[:, 0:1],
                        gt[:, q, :], ALU.mult, ALU.mult)
                    v = wpool.tile([128, 512], dt.bfloat16, tag="v")
                    nc.vector.tensor_tensor(v[:], t[:], xr[:, q, :], ALU.add)
                    if s == NS - 1:
                        nc.scalar.activation(OV[:, q, :], v[:], RELU)
                    else:
                        nc.vector.tensor_scalar(OV[:, q, :], v[:], 0.0, None, ALU.max)
                    if s == NS - 1 and q in (5, 6):
                        # drain early so the final store tail is short
                        q0 = 0 if q == 5 else 6
                        eng = nc.gpsimd if q == 5 else nc.sync
                        eng.dma_start(o_d[:, 8 * s + q0:8 * s + q + 1, :],
                                      OV[:, q0:q + 1, :])
                if s == NS - 1:
                    nc.scalar.dma_start(o_d[:, 8 * s + 7:8 * s + 8, :], OV[:, 7:8, :])
                else:
                    nc.gpsimd.dma_start(o_d[:, 8 * s:8 * s + 8, :], OV[:])

            def load_rg(s):
                xr = rgpool.tile([128, 8, 512], dt.bfloat16, tag="xr")
                gt = rgpool.tile([128, 8, 512], dt.float8e4, tag="gt")
                nc.sync.dma_start(xr[:], xr_d[:, 8 * s:8 * s + 8, :])
                nc.sync.dma_start(gt[:], gt_d[:, 8 * s:8 * s + 8, :])
                return xr, gt

            T1s = {0: emit_load(0, split=True)}
            nc.scalar.dma_start(sb1[:], sb1_d[:])
            nc.scalar.dma_start(sb2[:], sb2_d[:])
            nc.gpsimd.dma_start(wd2[:], wd2_d[:])
            T1s[1] = emit_load(1)
            H1s = {0: new_H1(), 1: new_H1()}
            rgs = {0: load_rg(0)}
            emit_conv1(0, T1s[0], H1s[0], H1s[1])
            for s in range(NS):
                if s + 2 < NS:
                    T1s[s + 2] = emit_load(s + 2)
                    H1s[s + 2] = new_H1()
                if s + 1 < NS:
                    rgs[s + 1] = load_rg(s + 1)
                    emit_conv1(s + 1, T1s[s + 1], H1s[s + 1], H1s.get(s + 2))
                emit_conv2(s, H1s[s], *rgs[s])
                T1s.pop(s, None)
                H1s.pop(s, None)
                rgs.pop(s, None)
    nc.finalize()
    return nc


def _host_prep(x, gate, w1, scale1, bias1, w2, scale2, bias2):
    # ---- weights: Wh = fp8(16w), Wl16 = fp8(256(w - Wh/16))/16, block-diag ----
    CP1 = [((0, 0), (1, 0)), ((0, 1), (2, 0)), ((1, 1), (2, 1)), ((0, 2), (1, 2))]
    CP2 = [((0, 2), (1, 2))]

    def pack_w(w, extra, cpairs):
        wt = np.transpose(w, (1, 0, 2, 3)).astype(np.float32)  # [ci, co, dy, dx]
        wh8 = (wt * KW).astype(FP8)
        wh = wh8.astype(np.float32)
        wl8 = ((wt * KW - wh) * KW).astype(FP8)
        wl16 = (wl8.astype(np.float32) / KW).astype(FP8).astype(np.float32)

        def bd(m):  # [64,64] -> block-diag [128,128]
            out = np.zeros((128, 128), np.float32)
            out[0:64, 0:64] = m
            out[64:128, 64:128] = m
            return out

        npair = 10 + len(cpairs)
        wd = np.zeros((128, npair, 2, 128), np.float32)
        for k in range(9):
            dy, dx = k // 3, k % 3
            m = bd(wh[:, :, dy, dx])
            wd[:, k, 0, :] = m
            wd[:, k, 1, :] = m
        for j, ((dy0, dx0), (dy1, dx1)) in enumerate(cpairs):
            wd[:, 9 + j, 0, :] = bd(wl16[:, :, dy0, dx0])
            wd[:, 9 + j, 1, :] = bd(wl16[:, :, dy1, dx1])
        wd[:, npair - 1, 0, :] = bd(wl16[:, :, 2, 2])
        wd[:, npair - 1, 1, :] = extra
        return wd.astype(FP8)

    wsel = np.zeros((128, 128), np.float32)
    wsel[0, 0:64] = BIGW
    wsel[1, 64:128] = BIGW
    s2 = scale2.astype(np.float32)
    wbias = np.zeros((128, 128), np.float32)
    wbias[0, 0:64] = KW * bias2 / s2
    wbias[1, 64:128] = KW * bias2 / s2
    wd1 = pack_w(w1, wsel, CP1)
    wd2 = pack_w(w2, wbias, CP2)

    sb1 = np.stack([np.tile(scale1 / KW, 2),
                    np.tile(bias1 - (BIGW / KW) * scale1, 2)], axis=1).astype(np.float32)
    sb2v = np.tile(s2 / KW, 2)[:, None].astype(np.float32)

    # ---- gate / gmax ----
    g = gate[:, 0]                                   # [B, H, W]
    gp = np.pad(g, ((0, 0), (1, 1), (1, 1)))
    gm = np.zeros_like(g)
    for dy in range(3):
        for dx in range(3):
            np.maximum(gm, gp[:, dy:dy + H, dx:dx + W], out=gm)

    xh8 = x.astype(FP8)                              # [B, C, H, W]
    xl8 = (x - xh8.astype(np.float32)).astype(FP8)

    xpack_l, xr_l, gt_l, = [], [], []
    # padded row lookup: prow = row + 2, rows [-2, 262)
    for bi in range(B):
        xhp = np.zeros((64, 264, WP), FP8)
        xlp = np.zeros((64, 264, WP), FP8)
        xhp[:, 2:2 + H, 1:257] = xh8[bi]
        xlp[:, 2:2 + H, 1:257] = xl8[bi]
        gmp = np.zeros((264, W), np.float32)
        gmp[2:2 + H] = gm[bi]
        xpack = np.zeros((128, NS * SRO, WP), FP8)
        for s in range(NS):
            r0 = R * s
            b0 = s * SRO
            xpack[0:64, b0:b0 + XRH] = xhp[:, r0:r0 + XRH]
            xpack[64:128, b0:b0 + XRH] = xhp[:, r0 + 2:r0 + 2 + XRH]
            xpack[0:64, b0 + XLO:b0 + XLO + XRH] = xlp[:, r0:r0 + XRH]
            xpack[64:128, b0 + XLO:b0 + XLO + XRH] = xlp[:, r0 + 2:r0 + 2 + XRH]
            for u in range(9):
                for hh in range(2):
                    t = r0 - 1 + 4 * u + hh
                    row = b0 + GOFF + 2 * u + hh
                    xpack[0, row, 0:256] = gmp[t + 2].astype(FP8)
                    xpack[1, row, 0:256] = gmp[t + 4].astype(FP8)
        xpack_l.append(xpack)

        xv = x[bi].reshape(64, NQ, 4, 256)
        xr = np.empty((128, NQ, 512), BF16)
        xr[0:64] = xv[:, :, 0:2].reshape(64, NQ, 512)
        xr[64:128] = xv[:, :, 2:4].reshape(64, NQ, 512)
        xr_l.append(xr)

        gv = g[bi].reshape(NQ, 4, 256)
        gt = np.empty((128, NQ, 512), FP8)
        gt[0:64] = np.broadcast_to(gv[:, 0:2].reshape(1, NQ, 512), (64, NQ, 512))
        gt[64:128] = np.broadcast_to(gv[:, 2:4].reshape(1, NQ, 512), (64, NQ, 512))
        gt_l.append(gt)

    return dict(wd1=wd1, wd2=wd2, sb1=sb1, sb2=sb2v,
                xp=xpack_l, xr=xr_l, gt=gt_l)


def _in_map(prep, bi):
    return {
        "xp": prep["xp"][bi], "xr": prep["xr"][bi], "gt": prep["gt"][bi],
        "wd1": prep["wd1"], "wd2": prep["wd2"],
        "sb1": prep["sb1"], "sb2": prep["sb2"],
    }


def kernel(x, gate, w1, scale1, bias1, w2, scale2, bias2):
    from concourse.bass_utils import run_bass_kernel_spmd

    x = np.asarray(x, np.float32)
    gate = np.asarray(gate, np.float32)
    prep = _host_prep(x, gate, np.asarray(w1, np.float32), np.asarray(scale1, np.float32),
                      np.asarray(bias1, np.float32), np.asarray(w2, np.float32),
                      np.asarray(scale2, np.float32), np.asarray(bias2, np.float32))

    if 'nc' not in _CACHE:
        _CACHE['nc'] = _build()
    nc = _CACHE['nc']

    in_maps = [_in_map(prep, bi) for bi in range(B)]
    res = run_bass_kernel_spmd(nc, in_maps, core_ids=list(range(B)))
    _CACHE['last_results'] = res
    out = np.empty((B, C, H, W), np.float32)
    for bi in range(B):
        ov = res.results[bi]["o"].astype(np.float32)    # [128, NQ, 512]
        ov4 = ov.reshape(128, NQ, 2, 256)
        out[bi, :, 0::4, :] = ov4[0:64, :, 0, :].transpose(0, 1, 2)
        out[bi, :, 1::4, :] = ov4[0:64, :, 1, :]
        out[bi, :, 2::4, :] = ov4[64:128, :, 0, :]
        out[bi, :, 3::4, :] = ov4[64:128, :, 1, :]
    return out
